# revision 34
# baseline (speedup 1.0000x reference)
"""Trainium2 Bass kernel for CausalSelfAttention (GQA + QK-RMSNorm + RoPE).

Problem shapes (hardcoded): B=2, S=2048, D=2048, H=16, KVH=4, HD=128.

Sharding: 8 cores = 2 batches x 4 kv-head groups. Core c handles batch
b = c // 4 and kv-group g = c % 4 (q-heads 4g..4g+3, kv head g).  Each core
computes its 4 heads end-to-end plus a partial output projection over its
512 columns of Wproj's input dim; the host sums the 4 partials per batch.

v2: all matmul operands bf16 (inputs DMA'd as bf16, outputs written bf16;
fp32 PSUM accumulation everywhere).  The softmax denominator is computed by
in-place bf16 accumulation of exp tiles in groups of 4 followed by a single
ones-matmul per group.  The phase-2 kt loop is software-pipelined so PE never
waits on ACT's exp; phase-3 (out-proj) for q-block qb is interleaved into the
attention heads of q-block qb+1 to fill PE gaps.
"""

import numpy as np

B, S, D = 2, 2048, 2048
H, KVH = 16, 4
HD = D // H            # 128
NH = H // KVH          # 4 heads per core
P = 128
ST = S // P            # 16 s-tiles
DT = D // P            # 16 d-tiles
FT = NH * HD // P      # 4 f-tiles (proj contraction per core)
QB = 512               # q-block width in phase 2
NQB = S // QB          # 4
SBW = 256              # phase-1 x DMA block width (s columns)
NSB = S // SBW         # 8 x-blocks
ROPE_BASE = 10000.0
EPS = 1e-6

_CACHE = {}


def _build_nc():
    from contextlib import ExitStack

    import concourse.mybir as mybir
    import concourse.tile as tile
    from concourse import bacc

    f32 = mybir.dt.float32
    bf16 = mybir.dt.bfloat16
    AF = mybir.ActivationFunctionType
    MUL = mybir.AluOpType.mult
    ADD = mybir.AluOpType.add

    nc = bacc.Bacc("TRN2", target_bir_lowering=False, debug=False, num_devices=8)

    xT = nc.dram_tensor("xT", [D, S], bf16, kind="ExternalInput").ap()
    wqT = nc.dram_tensor("wqT", [D, NH * HD], bf16, kind="ExternalInput").ap()
    wkvT = nc.dram_tensor("wkvT", [D, 2 * HD], bf16, kind="ExternalInput").ap()
    wpT = nc.dram_tensor("wpT", [NH * HD, D], bf16, kind="ExternalInput").ap()
    cs2 = nc.dram_tensor("cs2", [S, 2 * HD], bf16, kind="ExternalInput").ap()
    qg4 = nc.dram_tensor("qg4", [P, NH], f32, kind="ExternalInput").ap()
    trioi = nc.dram_tensor("trioi", [P, 3 * P], bf16, kind="ExternalInput").ap()
    outT = nc.dram_tensor("outT", [D, S], bf16, kind="ExternalOutput").ap()

    with tile.TileContext(nc) as tc:
        with ExitStack() as octx:
            const = octx.enter_context(tc.tile_pool(name="const", bufs=1))
            big = octx.enter_context(tc.tile_pool(name="big", bufs=1))

            # ---- persistent stores ----
            QT = big.tile([P, NH, S], bf16)   # q^T per head: [hd, h, s]
            KT = big.tile([P, S], bf16)       # k^T: [hd, s]
            VS = big.tile([P, ST, HD], bf16)  # v: [s-part, s-tile, hd]
            YT = big.tile([P, NH, S], bf16)   # attn out^T per head: [hd, h, s]
            WP = big.tile([P, FT, D], bf16)   # out-proj weights (prefetched)

            # PSUM pools shared across phases (same tag -> same slots, so
            # the phase boundary chains through WAR deps instead of a drain):
            #   ps_q  3 banks: phase-1 psq     + phase-2 score tiles
            #   ps_kv 2 banks: phase-1 pskv    + phase-3 po tiles
            #   ps_tr 3 banks: phase-1 transposes + per-head oT / den
            ps_q = octx.enter_context(
                tc.tile_pool(name="ps_q", bufs=3, space="PSUM"))
            ps_kv = octx.enter_context(
                tc.tile_pool(name="ps_kv", bufs=2, space="PSUM"))
            ps_tr = octx.enter_context(
                tc.tile_pool(name="ps_tr", bufs=3, space="PSUM"))

            # =========================== Phase 1 ===========================
            with ExitStack() as ctx1:
                wpool = ctx1.enter_context(tc.tile_pool(name="wpool", bufs=1))
                xpool = ctx1.enter_context(tc.tile_pool(name="xpool", bufs=3))
                stq = ctx1.enter_context(tc.tile_pool(name="stq", bufs=2))
                stk = ctx1.enter_context(tc.tile_pool(name="stk", bufs=2))
                sml = ctx1.enter_context(tc.tile_pool(name="sml", bufs=2))

                xTr = xT.rearrange("(dt p) s -> p dt s", p=P)
                wqTr = wqT.rearrange("(dt p) e -> p dt e", p=P)
                wkvTr = wkvT.rearrange("(dt p) e -> p dt e", p=P)

                WQ = wpool.tile([P, DT, NH * HD], bf16)
                WKV = wpool.tile([P, DT, 2 * HD], bf16)

                def load_xblk(sb):
                    t = xpool.tile([P, DT, SBW], bf16, tag="xblk", name="xblk")
                    nc.sync.dma_start(
                        t[:], xTr[:, :, sb * SBW:(sb + 1) * SBW])
                    return t

                # Startup: interleave WQ chunks with x-block-0 chunks so the
                # first Q matmuls' deps land earliest; WKV right after.
                # HWDGE pays ~625ns per DMA instruction, so chunks are coarse
                # (2/4/10 dt) — small first so PE starts early.
                xblk0 = xpool.tile([P, DT, SBW], bf16, tag="xblk", name="xblk")
                for c0, c1 in ((0, 2), (2, 6), (6, DT)):
                    nc.sync.dma_start(WQ[:, c0:c1, :], wqTr[:, c0:c1, :])
                    nc.sync.dma_start(
                        xblk0[:, c0:c1, :], xTr[:, c0:c1, 0:SBW])
                nc.sync.dma_start(WKV[:], wkvTr[:])
                xblk_next = load_xblk(1)

                # ---- constants ----
                cs_t = const.tile([P, ST, 2 * HD], bf16)  # [cos | sin] per s
                cs2r = cs2.rearrange("(st p) c -> p st c", p=P)
                nc.sync.dma_start(cs_t[:], cs2r[:])
                qg_t = const.tile([P, NH], f32)
                nc.sync.dma_start(qg_t[:], qg4)
                tio_t = const.tile([P, 3 * P], bf16)
                nc.sync.dma_start(tio_t[:], trioi)
                tri_t = tio_t[:, 0:P]
                ones_t = tio_t[:, P:2 * P]
                idb_t = tio_t[:, 2 * P:3 * P]
                eps_t = const.tile([P, 1], f32)
                nc.vector.memset(eps_t[:], EPS)

                xblk = xblk0

                nsb = SBW // P
                for sb in range(NSB):
                    if sb + 2 < NSB:
                        xblk_next2 = load_xblk(sb + 2)
                    if sb == 2:
                        nc.sync.dma_start(
                            WP[:], wpT.rearrange("(ft p) e -> p ft e", p=P))
                    for jj in range(nsb):
                        st = sb * nsb + jj
                        xs = xblk[:, :, jj * P:(jj + 1) * P]

                        psq = ps_q.tile([P, NH * HD], f32)
                        for dt in range(DT):
                            nc.tensor.matmul(
                                psq[:], xs[:, dt],
                                WQ[:, dt],
                                start=(dt == 0), stop=(dt == DT - 1))
                        pskv = ps_kv.tile([P, 2 * HD], f32)
                        for dt in range(DT):
                            nc.tensor.matmul(
                                pskv[:], xs[:, dt],
                                WKV[:, dt],
                                start=(dt == 0), stop=(dt == DT - 1))

                        # V straight to its store (ACT: frees pskv early)
                        nc.scalar.copy(VS[:, st], pskv[:, HD:2 * HD])

                        # -- sum of squares: one wide ACT square, segmented
                        # DVE reduce per head (keeps the ACT queue short so
                        # phase-2 exp isn't stuck behind rmsnorm backlog) --
                        ss5 = sml.tile([P, NH + 1], f32, tag="ss5")
                        psq3 = psq[:].rearrange("p (h c) -> p h c", h=NH)
                        sqscr = stq.tile([P, NH, HD], bf16, tag="sqj")
                        nc.scalar.activation(sqscr[:], psq3, AF.Square)
                        nc.vector.tensor_reduce(
                            ss5[:, 0:NH], sqscr[:], mybir.AxisListType.X,
                            ADD)
                        kjunk = stk.tile([P, HD], f32, tag="kjunk")
                        nc.scalar.activation(
                            kjunk[:], pskv[:, 0:HD], AF.Square,
                            accum_out=ss5[:, NH:NH + 1])

                        rms5 = sml.tile([P, NH + 1], f32, tag="rms5")
                        nc.scalar.activation(rms5[:], ss5[:], AF.Sqrt,
                                             bias=eps_t[:], scale=1.0 / HD)
                        ri5 = sml.tile([P, NH + 1], f32, tag="ri5")
                        nc.vector.reciprocal(ri5[:], rms5[:])
                        rsg = sml.tile([P, NH], f32, tag="rsg")
                        nc.vector.tensor_tensor(rsg[:], ri5[:, 0:NH], qg_t[:],
                                                MUL)

                        cos_st = cs_t[:, st, 0:HD]
                        sin_st = cs_t[:, st, HD:2 * HD]

                        # -- Q rmsnorm + rope + gain (DVE; bf16 after qn) --
                        qn = stq.tile([P, NH, HD], bf16, tag="qn")
                        nc.vector.tensor_tensor(
                            qn[:], psq3, rsg[:, :, None].to_broadcast([P, NH, HD]),
                            MUL)
                        qa = stq.tile([P, NH, HD], bf16, tag="qa")
                        nc.vector.tensor_tensor(
                            qa[:], qn[:],
                            cs_t[:, st:st + 1, 0:HD].to_broadcast([P, NH, HD]),
                            MUL)
                        qb = stq.tile([P, NH, HD], bf16, tag="qb")
                        nc.vector.tensor_tensor(
                            qb[:, :, 0:HD // 2], qn[:, :, HD // 2:HD],
                            cs_t[:, st:st + 1, HD:HD + HD // 2].to_broadcast(
                                [P, NH, HD // 2]), MUL)
                        nc.vector.tensor_tensor(
                            qb[:, :, HD // 2:HD], qn[:, :, 0:HD // 2],
                            cs_t[:, st:st + 1, HD + HD // 2:2 * HD].to_broadcast(
                                [P, NH, HD // 2]), MUL)
                        qrot = stq.tile([P, NH, HD], bf16, tag="qn")
                        nc.vector.tensor_tensor(qrot[:], qa[:], qb[:], ADD)

                        # -- K rmsnorm + rope (Pool) --
                        kn = stk.tile([P, HD], bf16, tag="kn")
                        nc.vector.tensor_tensor(
                            kn[:], pskv[:, 0:HD],
                            ri5[:, NH:NH + 1].to_broadcast([P, HD]), MUL)
                        ka = stk.tile([P, HD], bf16, tag="ka")
                        nc.gpsimd.tensor_tensor(ka[:], kn[:], cos_st, MUL)
                        kb = stk.tile([P, HD], bf16, tag="kb")
                        nc.gpsimd.tensor_tensor(
                            kb[:, 0:HD // 2], kn[:, HD // 2:HD],
                            sin_st[:, 0:HD // 2], MUL)
                        nc.gpsimd.tensor_tensor(
                            kb[:, HD // 2:HD], kn[:, 0:HD // 2],
                            sin_st[:, HD // 2:HD], MUL)
                        krot = stk.tile([P, HD], bf16, tag="kn")
                        nc.gpsimd.tensor_tensor(krot[:], ka[:], kb[:], ADD)

                        # -- transposes into QT / KT (bf16: 1 cycle/row) --
                        for hh in range(NH):
                            ptr = ps_tr.tile([P, P], bf16, tag="tr")
                            nc.tensor.transpose(ptr[:], qrot[:, hh], idb_t)
                            nc.vector.tensor_copy(
                                QT[:, hh, st * P:(st + 1) * P], ptr[:])
                        ptrk = ps_tr.tile([P, P], bf16, tag="tr")
                        nc.tensor.transpose(ptrk[:], krot[:], idb_t)
                        nc.vector.tensor_copy(KT[:, st * P:(st + 1) * P],
                                              ptrk[:])
                    if sb + 1 < NSB:
                        xblk = xblk_next
                    if sb + 2 < NSB:
                        xblk_next = xblk_next2

            # ======================= Phases 2 and 3 ========================
            with ExitStack() as ctx2:
                ostage = ctx2.enter_context(tc.tile_pool(name="ostage", bufs=6))

                with ExitStack() as ctx2b:
                    expool = ctx2b.enter_context(
                        tc.tile_pool(name="expool", bufs=14))
                    recpool = ctx2b.enter_context(
                        tc.tile_pool(name="recpool", bufs=3))

                    # phase-3 emitter: 4 consecutive et column-blocks of the
                    # partial out-projection for s-block sb3, staged to one
                    # SBUF tile and written with a single 512-row DMA.
                    outTr = outT.rearrange("(e p) s -> p e s", p=P)

                    def emit_p3(sb3, et0, nets=4, use_act=False):
                        ob4 = ostage.tile([P, nets, QB], bf16, tag="ob",
                                          name="ob")
                        for i in range(nets):
                            et = et0 + i
                            po = ps_kv.tile([P, QB], f32, tag="pskv",
                                            name="po")
                            for ft in range(FT):
                                nc.tensor.matmul(
                                    po[:],
                                    WP[:, ft, et * P:(et + 1) * P],
                                    YT[:, ft, sb3 * QB:(sb3 + 1) * QB],
                                    start=(ft == 0), stop=(ft == FT - 1))
                            if use_act and i % 2 == 0:
                                nc.scalar.copy(ob4[:, i], po[:])
                            else:
                                nc.vector.tensor_copy(ob4[:, i], po[:])
                        nc.sync.dma_start(
                            outTr[:, et0:et0 + nets,
                                  sb3 * QB:(sb3 + 1) * QB], ob4[:])

                    for qb in range(NQB):
                        # phase-3 work of the previous q-block, interleaved
                        # after each head of this q-block
                        p3q = list(range(DT // 4)) if qb > 0 else []

                        for h in range(NH):
                            oT = ps_tr.tile([P, QB], f32, tag="tr", name="oT")
                            den = ps_tr.tile([P, QB], f32, tag="tr",
                                             name="den")
                            nk = NH * qb + NH

                            # software pipeline: scores+exp run 2 tiles ahead
                            # of PV so PE never waits on ACT's exp latency.
                            DEPTH = 2
                            exs = {}
                            leader = None

                            def emit_front(kt):
                                j = kt - NH * qb
                                q0 = P * j if j >= 0 else 0
                                ps = ps_q.tile([P, QB], f32, tag="psq",
                                               name="ps")
                                nc.tensor.matmul(
                                    ps[:, q0:QB],
                                    KT[:, kt * P:(kt + 1) * P],
                                    QT[:, h, qb * QB + q0:(qb + 1) * QB],
                                    start=True, stop=True)
                                ex = expool.tile([P, QB], bf16, tag="ex",
                                                 name="ex")
                                nc.scalar.activation(
                                    ex[:, q0:QB], ps[:, q0:QB], AF.Exp)
                                if j >= 0:
                                    nc.vector.tensor_tensor(
                                        ex[:, q0:q0 + P], ex[:, q0:q0 + P],
                                        tri_t, MUL)
                                exs[kt] = (ex, q0)

                            def emit_back(kt):
                                nonlocal leader
                                ex, q0 = exs.pop(kt)
                                nc.tensor.matmul(
                                    oT[:, q0:QB], VS[:, kt], ex[:, q0:QB],
                                    start=(kt == 0), stop=(kt == nk - 1))
                                # denominator: in-place bf16 accumulation in
                                # groups of 8, one ones-matmul per group
                                if kt % 8 == 0:
                                    leader = ex
                                else:
                                    nc.vector.tensor_tensor(
                                        leader[:, q0:QB], leader[:, q0:QB],
                                        ex[:, q0:QB], ADD)
                                if kt % 8 == 7 or kt == nk - 1:
                                    nc.tensor.matmul(
                                        den[:], ones_t, leader[:],
                                        start=(kt < 8), stop=(kt == nk - 1))

                            for kt in range(nk):
                                emit_front(kt)
                                if kt >= DEPTH:
                                    emit_back(kt - DEPTH)
                                # one phase-3 block per head fills PE gaps
                                if p3q and kt == 1:
                                    emit_p3(qb - 1, 4 * p3q.pop(0))
                            for kt in range(nk - DEPTH, nk):
                                emit_back(kt)

                            rec = recpool.tile([P, QB], f32, tag="rec")
                            nc.vector.reciprocal_approx_fast(rec[:], den[:])
                            nc.vector.tensor_tensor(
                                YT[:, h, qb * QB:(qb + 1) * QB], oT[:], rec[:],
                                MUL)
                        # leftover phase-3 blocks of the previous q-block
                        for eb in p3q:
                            emit_p3(qb - 1, 4 * eb)

                    # tail: phase-3 of the last q-block; shallow staging and
                    # two copy engines so the final DMA chain stays short
                    for eb2 in range(DT // 2):
                        emit_p3(NQB - 1, 2 * eb2, nets=2, use_act=True)

    nc.compile()
    return nc


def _host_inputs(x, Wq, Wk, Wv, Wproj, q_gain):
    """Build the 8 per-core input maps (all big operands as bf16)."""
    import ml_dtypes
    f32 = np.float32
    bf16 = ml_dtypes.bfloat16
    inv_freq = 1.0 / (ROPE_BASE ** (np.arange(0, HD, 2, dtype=f32) / HD))
    freqs = np.outer(np.arange(S, dtype=f32), inv_freq).astype(f32)
    c = np.cos(freqs).astype(f32)
    s = np.sin(freqs).astype(f32)
    # [cos|cos|sin|-sin] packed: cs2[:, 0:HD] = cos2, cs2[:, HD:] = sin2
    cs2 = np.concatenate([c, c, s, -s], axis=1).astype(bf16)
    tri = np.triu(np.ones((P, P), dtype=f32))          # tri[k, q] = k <= q
    onesd = np.ones((P, P), dtype=f32)
    ident = np.eye(P, dtype=f32)
    trioi = np.concatenate([tri, onesd, ident], axis=1).astype(bf16)

    in_maps = []
    for core in range(8):
        b, g = divmod(core, KVH)
        hs = g * NH * HD            # first q row for this group
        qg = (q_gain[g * NH:(g + 1) * NH].astype(f32) * (HD ** -0.5))
        in_maps.append({
            "xT": np.ascontiguousarray(x[b].T).astype(bf16),
            "wqT": np.ascontiguousarray(Wq[hs:hs + NH * HD].T).astype(bf16),
            "wkvT": np.ascontiguousarray(
                np.concatenate([Wk[g * HD:(g + 1) * HD], Wv[g * HD:(g + 1) * HD]],
                               axis=0).T).astype(bf16),
            "wpT": np.ascontiguousarray(Wproj.T[hs:hs + NH * HD]).astype(bf16),
            "cs2": cs2,
            "qg4": np.ascontiguousarray(np.broadcast_to(qg, (P, NH)), dtype=f32),
            "trioi": trioi,
        })
    return in_maps


def kernel(x, Wq, Wk, Wv, Wproj, q_gain):
    from concourse.bass_utils import run_bass_kernel_spmd

    x = np.asarray(x, dtype=np.float32)
    Wq = np.asarray(Wq, dtype=np.float32)
    Wk = np.asarray(Wk, dtype=np.float32)
    Wv = np.asarray(Wv, dtype=np.float32)
    Wproj = np.asarray(Wproj, dtype=np.float32)
    q_gain = np.asarray(q_gain, dtype=np.float32)

    if "nc" not in _CACHE:
        _CACHE["nc"] = _build_nc()
    nc = _CACHE["nc"]

    in_maps = _host_inputs(x, Wq, Wk, Wv, Wproj, q_gain)
    res = run_bass_kernel_spmd(nc, in_maps, core_ids=list(range(8)))

    out = np.zeros((B, S, D), dtype=np.float32)
    for core in range(8):
        b = core // KVH
        out[b] += np.asarray(res.results[core]["outT"]).astype(np.float32).T
    return out
